# revision 8
# baseline (speedup 1.0000x reference)
"""Trainium2 Bass kernel for nn_MLPModel_70703751626902 (moe_routing).

Per-robot hypernetwork MLP: each of 1024 samples routes to one of 32
per-robot weight sets (input hypernet 624->256, three 256x256 hidden
layers, output hypernet 256->24).

Strategy (expert-parallel): group samples by robot on the host, shard
robots across the 8 cores (4 robots/core, one per "slot"), so every
core runs dense per-robot matmuls with only its own robots' weights
(~5.8MB/core instead of 46MB replicated). Activations stay transposed
([hidden, batch]) the whole way so each layer's PSUM output feeds the
next layer's moving operand directly. Biases are folded in as K=1 (or
K=12 for the masked input bias) matmuls into the same PSUM
accumulation group. The obs mask is applied with one elementwise
multiply on the transposed input.

Samples for slot j occupy columns [off_j, off_j + cap_j) where cap_j is
the max sample count over the 8 robots assigned to slot j (rounded up
to 8); robots are assigned to slots by descending count so the padding
waste is small. All 8 cores run an identical program (SPMD).
"""

import numpy as np

F32 = np.float32


def _plan(ids, n_robots, obs_len_b):
    """Group samples by robot and assign robots to (core, slot)."""
    counts = np.bincount(ids, minlength=n_robots)
    order = np.argsort(-counts, kind="stable")  # robots by count desc
    n_slots = (n_robots + 7) // 8
    caps = []
    for j in range(n_slots):
        grp = order[8 * j : 8 * j + 8]
        m = int(counts[grp].max()) if len(grp) else 0
        caps.append(max(8, int(np.ceil(max(m, 1) / 8) * 8)))
    offs = np.concatenate([[0], np.cumsum(caps)]).astype(int)
    nb = int(offs[-1])
    assert nb <= 512, f"batch columns per core {nb} exceeds PSUM bank"
    # rows[core][slot] = sample indices for that robot (order preserved)
    rows = [[None] * n_slots for _ in range(8)]
    robot_at = [[None] * n_slots for _ in range(8)]
    for rank, robot in enumerate(order):
        j, c = rank // 8, rank % 8
        if j >= n_slots:
            break
        rows[c][j] = np.nonzero(ids == robot)[0]
        robot_at[c][j] = int(robot)
    return {
        "caps": tuple(caps),
        "offs": tuple(int(o) for o in offs),
        "nb": nb,
        "rows": rows,
        "robot_at": robot_at,
        "n_slots": n_slots,
    }


_PROGRAM_CACHE = {}


def _build_program(caps, kin, hid, kout):
    """Emit the per-core Bass/Tile program (identical on all 8 cores)."""
    import concourse.mybir as mybir
    import concourse.tile as tile
    from concourse import bacc

    dt = mybir.dt.float32
    n_slots = len(caps)
    offs = np.concatenate([[0], np.cumsum(caps)]).astype(int)
    nb = int(offs[-1])
    ksz = [128] * (kin // 128) + ([kin % 128] if kin % 128 else [])
    nh = hid // 128  # hidden column halves (2 for hid=256)
    maxcap = max(caps)

    nc = bacc.Bacc("TRN2", target_bir_lowering=False, debug=False, num_devices=8)

    xt_d = nc.dram_tensor("xt", [kin, nb], dt, kind="ExternalInput")
    me_d = nc.dram_tensor("mexp", [kin, nb], dt, kind="ExternalInput")
    mb_d = nc.dram_tensor("mbart", [12, nb], dt, kind="ExternalInput")
    wi_d = nc.dram_tensor("wi", [n_slots, kin, hid], dt, kind="ExternalInput")
    bi_d = nc.dram_tensor("bi", [n_slots, 12, hid], dt, kind="ExternalInput")
    w_d = [
        nc.dram_tensor(f"w{li}", [n_slots, hid, hid], dt, kind="ExternalInput")
        for li in (1, 2, 3)
    ]
    wo_d = nc.dram_tensor("wo", [n_slots, hid, kout], dt, kind="ExternalInput")
    bv_d = nc.dram_tensor("bvec", [4, n_slots * hid], dt, kind="ExternalInput")
    ot_d = nc.dram_tensor("ot", [kout, nb], dt, kind="ExternalOutput")

    relu = mybir.ActivationFunctionType.Relu

    with tile.TileContext(nc) as tc:
        with (
            tc.tile_pool(name="sb", bufs=1) as pool,
            tc.tile_pool(name="ps", bufs=4, space="PSUM") as psum,
            tc.tile_pool(name="pso", bufs=1, space="PSUM") as psum_o,
        ):
            ones = pool.tile([1, maxcap], dt, tag="ones")
            nc.gpsimd.memset(ones[:], 1.0)
            mb_t = pool.tile([12, nb], dt, tag="mbart")
            nc.sync.dma_start(mb_t[:], mb_d[:, :])

            # masked transposed input: xm[kt] = xt[kt] * mexp[kt]
            xm = []
            for kt, ks in enumerate(ksz):
                r0 = kt * 128
                xt_t = pool.tile([ks, nb], dt, tag=f"xt{kt}")
                me_t = pool.tile([ks, nb], dt, tag=f"me{kt}")
                xm_t = pool.tile([ks, nb], dt, tag=f"xm{kt}")
                nc.sync.dma_start(xt_t[:], xt_d[r0 : r0 + ks, :])
                nc.sync.dma_start(me_t[:], me_d[r0 : r0 + ks, :])
                nc.vector.tensor_mul(xm_t[:], xt_t[:], me_t[:])
                xm.append(xm_t)

            # bias rows: one [1, n_slots*hid] tile per layer so every
            # lhsT slice starts at partition 0 (hw base-partition rule)
            bv_t = []
            for li in range(4):
                t = pool.tile([1, n_slots * hid], dt, tag=f"bv{li}")
                nc.sync.dma_start(t[:], bv_d[li : li + 1, :])
                bv_t.append(t)

            # weight tiles
            wi_t, bi_t = [], []
            w_t = [[], [], []]
            wo_t = []
            for j in range(n_slots):
                wi_t.append([])
                for kt, ks in enumerate(ksz):
                    t = pool.tile([ks, hid], dt, tag=f"wi{j}k{kt}")
                    nc.sync.dma_start(t[:], wi_d[j, kt * 128 : kt * 128 + ks, :])
                    wi_t[j].append(t)
                t = pool.tile([12, hid], dt, tag=f"bi{j}")
                nc.sync.dma_start(t[:], bi_d[j, :, :])
                bi_t.append(t)
                for li in range(3):
                    halves = []
                    for p in range(nh):
                        t = pool.tile([128, hid], dt, tag=f"w{li}j{j}p{p}")
                        nc.sync.dma_start(t[:], w_d[li][j, p * 128 : p * 128 + 128, :])
                        halves.append(t)
                    w_t[li].append(halves)
                halves = []
                for p in range(nh):
                    t = pool.tile([128, kout], dt, tag=f"wo{j}p{p}")
                    nc.sync.dma_start(t[:], wo_d[j, p * 128 : p * 128 + 128, :])
                    halves.append(t)
                wo_t.append(halves)

            def col(j):
                return slice(int(offs[j]), int(offs[j]) + caps[j])

            # Layer 0: act0[h, b] = relu(Wi^T xm + (1-mask) @ bi)
            act0 = pool.tile([128, nh * nb], dt, tag="act0")
            for h in range(nh):
                p0 = psum.tile([128, nb], dt, tag="ps")
                hs = slice(h * 128, h * 128 + 128)
                for j in range(n_slots):
                    sl = col(j)
                    for kt in range(len(ksz)):
                        nc.tensor.matmul(
                            p0[:, sl], wi_t[j][kt][:, hs], xm[kt][:, sl],
                            start=(kt == 0), stop=False,
                        )
                    nc.tensor.matmul(
                        p0[:, sl], bi_t[j][:, hs], mb_t[:, sl],
                        start=False, stop=True,
                    )
                dst = act0[:, h * nb : (h + 1) * nb]
                if h % 2 == 0:
                    nc.scalar.activation(dst, p0[:], relu)
                else:
                    nc.vector.tensor_relu(dst, p0[:])

            # Hidden layers 1-3
            prev = act0
            for li in range(3):
                nxt = pool.tile([128, nh * nb], dt, tag=f"act{li + 1}")
                for h in range(nh):
                    p = psum.tile([128, nb], dt, tag="ps")
                    hs = slice(h * 128, h * 128 + 128)
                    for j in range(n_slots):
                        sl = col(j)
                        for p_in in range(nh):
                            nc.tensor.matmul(
                                p[:, sl],
                                w_t[li][j][p_in][:, hs],
                                prev[:, p_in * nb + int(offs[j]) : p_in * nb + int(offs[j]) + caps[j]],
                                start=(p_in == 0), stop=False,
                            )
                        nc.tensor.matmul(
                            p[:, sl],
                            bv_t[li][0:1, j * hid + h * 128 : j * hid + h * 128 + 128],
                            ones[:, : caps[j]],
                            start=False, stop=True,
                        )
                    dst = nxt[:, h * nb : (h + 1) * nb]
                    if h % 2 == 0:
                        nc.scalar.activation(dst, p[:], relu)
                    else:
                        nc.vector.tensor_relu(dst, p[:])
                prev = nxt

            # Output layer (no relu)
            po = psum_o.tile([kout, nb], dt, tag="po")
            for j in range(n_slots):
                sl = col(j)
                for p_in in range(nh):
                    nc.tensor.matmul(
                        po[:, sl],
                        wo_t[j][p_in][:, :],
                        prev[:, p_in * nb + int(offs[j]) : p_in * nb + int(offs[j]) + caps[j]],
                        start=(p_in == 0), stop=False,
                    )
                nc.tensor.matmul(
                    po[:, sl], bv_t[3][0:1, j * hid : j * hid + kout],
                    ones[:, : caps[j]],
                    start=False, stop=True,
                )
            ot_t = pool.tile([kout, nb], dt, tag="ot")
            nc.scalar.activation(ot_t[:], po[:], mybir.ActivationFunctionType.Copy)
            nc.sync.dma_start(ot_d[:, :], ot_t[:])

    nc.compile()
    return nc


def _get_program(caps, kin, hid, kout):
    key = (caps, kin, hid, kout)
    if key not in _PROGRAM_CACHE:
        _PROGRAM_CACHE[key] = _build_program(caps, kin, hid, kout)
    return _PROGRAM_CACHE[key]


def _prep_core_inputs(plan, c, obs, maskbar, Wi, bi, W1, b1, W2, b2, W3, b3, Wo, bo):
    seq = maskbar.shape[1]
    lobs = obs.shape[1] // seq
    kin = obs.shape[1]
    hid = Wi.shape[3]
    kout = seq * Wo.shape[3]
    n_slots = plan["n_slots"]
    nb = plan["nb"]
    offs = plan["offs"]

    xt = np.zeros((kin, nb), F32)
    mexp = np.zeros((kin, nb), F32)
    mbart = np.zeros((seq, nb), F32)
    wi = np.zeros((n_slots, kin, hid), F32)
    bi_a = np.zeros((n_slots, seq, hid), F32)
    w = [np.zeros((n_slots, hid, hid), F32) for _ in range(3)]
    wo = np.zeros((n_slots, hid, kout), F32)
    bv = np.zeros((4, n_slots * hid), F32)

    for j in range(n_slots):
        r = plan["robot_at"][c][j]
        if r is None:
            continue
        rows = plan["rows"][c][j]
        n = len(rows)
        o0 = offs[j]
        if n:
            xt[:, o0 : o0 + n] = obs[rows].T
            mb = maskbar[rows]  # [n, seq]
            mexp[:, o0 : o0 + n] = np.repeat(mb, lobs, axis=1).T
            mbart[:, o0 : o0 + n] = mb.T
        wi[j] = Wi[r].reshape(kin, hid)
        bi_a[j] = bi[r]
        w[0][j], w[1][j], w[2][j] = W1[r], W2[r], W3[r]
        wo[j] = Wo[r].transpose(1, 0, 2).reshape(hid, kout)
        h0 = j * hid
        bv[0, h0 : h0 + hid] = b1[r]
        bv[1, h0 : h0 + hid] = b2[r]
        bv[2, h0 : h0 + hid] = b3[r]
        bv[3, h0 : h0 + kout] = bo[r].reshape(-1)

    return {
        "xt": xt, "mexp": mexp, "mbart": mbart, "wi": wi, "bi": bi_a,
        "w1": w[0], "w2": w[1], "w3": w[2], "wo": wo, "bvec": bv,
    }


def _unshard(plan, results, B, kout):
    out = np.zeros((B, kout), F32)
    offs = plan["offs"]
    for c in range(8):
        ot = results[c]["ot"]  # [kout, nb]
        for j in range(plan["n_slots"]):
            rows = plan["rows"][c][j]
            if rows is None or len(rows) == 0:
                continue
            o0 = offs[j]
            out[rows] = ot[:, o0 : o0 + len(rows)].T
    return out


def kernel(obs, obs_mask, unimal_ids, Wi, bi, W1, b1, W2, b2, W3, b3, Wo, bo,
           _runner=None):
    obs = np.asarray(obs, F32)
    obs_mask = np.asarray(obs_mask)
    ids = np.asarray(unimal_ids).astype(np.int64)
    Wi, bi = np.asarray(Wi, F32), np.asarray(bi, F32)
    W1, b1 = np.asarray(W1, F32), np.asarray(b1, F32)
    W2, b2 = np.asarray(W2, F32), np.asarray(b2, F32)
    W3, b3 = np.asarray(W3, F32), np.asarray(b3, F32)
    Wo, bo = np.asarray(Wo, F32), np.asarray(bo, F32)

    B = obs.shape[0]
    n_robots = Wi.shape[0]
    seq, lobs, hid = Wi.shape[1], Wi.shape[2], Wi.shape[3]
    kin = seq * lobs
    kout = seq * Wo.shape[3]
    maskbar = 1.0 - obs_mask.astype(F32)

    plan = _plan(ids, n_robots, kin)
    nc = _get_program(plan["caps"], kin, hid, kout)

    in_maps = [
        _prep_core_inputs(plan, c, obs, maskbar, Wi, bi, W1, b1, W2, b2, W3, b3, Wo, bo)
        for c in range(8)
    ]

    if _runner is None:
        from concourse.bass_utils import run_bass_kernel_spmd

        res = run_bass_kernel_spmd(nc, in_maps, core_ids=list(range(8)))
        results = res.results
    else:
        results = _runner(nc, in_maps)

    return _unshard(plan, results, B, kout)


# revision 11
# speedup vs baseline: 1.2324x; 1.2324x over previous
"""Trainium2 Bass kernel for nn_MLPModel_70703751626902 (moe_routing).

Per-robot hypernetwork MLP: each of 1024 samples routes to one of 32
per-robot weight sets (input hypernet 624->256, three 256x256 hidden
layers, output hypernet 256->24).

Strategy (expert-parallel): group samples by robot on the host, shard
robots across the 8 cores (4 robots/core, one per "slot"), so every
core runs dense per-robot matmuls with only its own robots' weights
(~5.8MB/core instead of 46MB replicated). Activations stay transposed
([hidden, batch]) the whole way so each layer's PSUM output feeds the
next layer's moving operand directly. The obs mask is folded into the
transposed input with one elementwise multiply; the masked input bias
(maskbar @ bi) is a K=12 matmul into the same PSUM accumulation
group; all other biases ride along as per-partition bias operands of
the PSUM->SBUF relu/copy activation ops.

All DRAM tensors are packed host-side so every DMA moves >=2KB
contiguous runs per partition (128-partition-major packing of the
contraction dim).

Samples for slot j occupy columns [off_j, off_j + cap_j) where cap_j is
the max sample count over the 8 robots assigned to slot j (rounded up
to 8); robots are assigned to slots by descending count so padding
waste is small. All 8 cores run an identical program (SPMD).
"""

import numpy as np

F32 = np.float32

# matmul operand dtype: "f32" (exact), "f32r" (fp32 bits, faster PE
# mode, slightly relaxed numerics), "bf16" (half DMA bytes, 4x PE rate)
W_DT = "f32"


def _plan(ids, n_robots):
    """Group samples by robot and assign robots to (core, slot)."""
    counts = np.bincount(ids, minlength=n_robots)
    order = np.argsort(-counts, kind="stable")
    n_slots = (n_robots + 7) // 8
    caps = []
    for j in range(n_slots):
        grp = order[8 * j : 8 * j + 8]
        m = int(counts[grp].max()) if len(grp) else 0
        caps.append(max(8, int(np.ceil(max(m, 1) / 8) * 8)))
    offs = np.concatenate([[0], np.cumsum(caps)]).astype(int)
    nb = int(offs[-1])
    assert nb <= 512, f"batch columns per core {nb} exceeds PSUM bank"
    rows = [[None] * n_slots for _ in range(8)]
    robot_at = [[None] * n_slots for _ in range(8)]
    for rank, robot in enumerate(order):
        j, c = rank // 8, rank % 8
        if j >= n_slots:
            break
        rows[c][j] = np.nonzero(ids == robot)[0]
        robot_at[c][j] = int(robot)
    return {
        "caps": tuple(caps),
        "offs": tuple(int(o) for o in offs),
        "nb": nb,
        "rows": rows,
        "robot_at": robot_at,
        "n_slots": n_slots,
    }


def _pack_kp(a, ncols=None):
    """[K, M] -> [128, ceil(K/128)*M]; col kt*M+m holds a[kt*128+p, m]."""
    k, m = a.shape
    nk = (k + 127) // 128
    out = np.zeros((128, nk * m), a.dtype)
    for kt in range(nk):
        ks = min(128, k - kt * 128)
        out[:ks, kt * m : kt * m + m] = a[kt * 128 : kt * 128 + ks, :]
    return out


_PROGRAM_CACHE = {}


def _build_program(caps, kin, hid, kout, w_dt_name):
    import concourse.mybir as mybir
    import concourse.tile as tile
    from concourse import bacc

    f32 = mybir.dt.float32
    wdt = {"f32": f32, "f32r": mybir.dt.float32r, "bf16": mybir.dt.bfloat16}[w_dt_name]
    n_slots = len(caps)
    offs = np.concatenate([[0], np.cumsum(caps)]).astype(int)
    nb = int(offs[-1])
    nk = (kin + 127) // 128  # 5 contraction chunks for the input layer
    klast = kin - 128 * (nk - 1)
    nh = hid // 128  # hidden column halves

    nc = bacc.Bacc("TRN2", target_bir_lowering=False, debug=False, num_devices=8)

    xt_d = nc.dram_tensor("xt", [128, nk * nb], wdt, kind="ExternalInput")
    me_d = nc.dram_tensor("mexp", [128, nk * nb], wdt, kind="ExternalInput")
    mb_d = nc.dram_tensor("mbart", [12, nb], wdt, kind="ExternalInput")
    bi_d = nc.dram_tensor("bi", [12, n_slots * hid], wdt, kind="ExternalInput")
    bc_d = nc.dram_tensor("bcols", [128, n_slots * 8], f32, kind="ExternalInput")
    wi_d = nc.dram_tensor("wi", [n_slots, 128, nk * hid], wdt, kind="ExternalInput")
    w_d = [
        nc.dram_tensor(f"w{li}", [n_slots, 128, nh * hid], wdt, kind="ExternalInput")
        for li in (1, 2, 3)
    ]
    wo_d = nc.dram_tensor("wo", [n_slots, 128, nh * kout], wdt, kind="ExternalInput")
    ot_d = nc.dram_tensor("ot", [kout, nb], f32, kind="ExternalOutput")

    relu = mybir.ActivationFunctionType.Relu
    ident = mybir.ActivationFunctionType.Identity
    act_parity = [0]

    with tile.TileContext(nc) as tc:
        with (
            tc.tile_pool(name="sb", bufs=1) as pool,
            tc.tile_pool(name="ps", bufs=4, space="PSUM") as psum,
            tc.tile_pool(name="pso", bufs=1, space="PSUM") as psum_o,
        ):
            # small tensors first
            bc_t = pool.tile([128, n_slots * 8], f32, tag="bc")
            nc.sync.dma_start(bc_t[:], bc_d[:, :])
            mb_t = pool.tile([12, nb], wdt, tag="mbart")
            nc.sync.dma_start(mb_t[:], mb_d[:, :])
            bi_t = pool.tile([12, n_slots * hid], wdt, tag="bi")
            nc.sync.dma_start(bi_t[:], bi_d[:, :])

            # masked transposed input (single packed multiply)
            xt_t = pool.tile([128, nk * nb], wdt, tag="xt")
            me_t = pool.tile([128, nk * nb], wdt, tag="me")
            xm_t = pool.tile([128, nk * nb], wdt, tag="xm")
            nc.sync.dma_start(xt_t[:], xt_d[:, :])
            nc.sync.dma_start(me_t[:], me_d[:, :])
            nc.vector.tensor_mul(xm_t[:], xt_t[:], me_t[:])

            wi_t = []
            for j in range(n_slots):
                t = pool.tile([128, nk * hid], wdt, tag=f"wi{j}")
                nc.sync.dma_start(t[:], wi_d[j, :, :])
                wi_t.append(t)
            w_t = [[], [], []]
            for li in range(3):
                for j in range(n_slots):
                    t = pool.tile([128, nh * hid], wdt, tag=f"w{li}j{j}")
                    nc.sync.dma_start(t[:], w_d[li][j, :, :])
                    w_t[li].append(t)
            wo_t = []
            for j in range(n_slots):
                t = pool.tile([128, nh * kout], wdt, tag=f"wo{j}")
                nc.sync.dma_start(t[:], wo_d[j, :, :])
                wo_t.append(t)

            def act_op(dst, src, func, bias):
                """PSUM->SBUF activation, alternating scalar/vector engines."""
                if act_parity[0] % 2 == 0:
                    nc.scalar.activation(dst, src, func, bias=bias)
                elif func is relu:
                    nc.vector.tensor_scalar(
                        dst, src, bias, 0.0,
                        mybir.AluOpType.add, mybir.AluOpType.max,
                    )
                else:
                    nc.vector.tensor_scalar(
                        dst, src, bias, None, mybir.AluOpType.add,
                    )
                act_parity[0] += 1

            # Layer 0: act0[h, b] = relu(Wi^T xm + maskbar @ bi)
            act0 = pool.tile([128, nh * nb], wdt, tag="act0")
            p0 = [psum.tile([128, nb], f32, tag="ps", name=f"p0h{h}") for h in range(nh)]
            for j in range(n_slots):
                sl = slice(int(offs[j]), int(offs[j]) + caps[j])
                for h in range(nh):
                    hs = slice(h * 128, h * 128 + 128)
                    for kt in range(nk):
                        ks = 128 if kt < nk - 1 else klast
                        nc.tensor.matmul(
                            p0[h][:, sl],
                            wi_t[j][:ks, kt * hid + h * 128 : kt * hid + h * 128 + 128],
                            xm_t[:ks, kt * nb + int(offs[j]) : kt * nb + int(offs[j]) + caps[j]],
                            start=(kt == 0), stop=False,
                        )
                    nc.tensor.matmul(
                        p0[h][:, sl],
                        bi_t[:, j * hid + h * 128 : j * hid + h * 128 + 128],
                        mb_t[:, sl],
                        start=False, stop=True,
                    )
            for h in range(nh):
                act_op(act0[:, h * nb : (h + 1) * nb], p0[h][:, :], relu, 0.0)

            # Hidden layers
            prev = act0
            for li in range(3):
                nxt = pool.tile([128, nh * nb], wdt, tag=f"act{li + 1}")
                for h in range(nh):
                    p = psum.tile([128, nb], f32, tag="ps")
                    for j in range(n_slots):
                        sl = slice(int(offs[j]), int(offs[j]) + caps[j])
                        for pi in range(nh):
                            nc.tensor.matmul(
                                p[:, sl],
                                w_t[li][j][:, pi * hid + h * 128 : pi * hid + h * 128 + 128],
                                prev[:, pi * nb + int(offs[j]) : pi * nb + int(offs[j]) + caps[j]],
                                start=(pi == 0), stop=(pi == nh - 1),
                            )
                    for j in range(n_slots):
                        sl = slice(int(offs[j]), int(offs[j]) + caps[j])
                        bias = bc_t[:, j * 8 + li * 2 + h : j * 8 + li * 2 + h + 1]
                        act_op(
                            nxt[:, h * nb + int(offs[j]) : h * nb + int(offs[j]) + caps[j]],
                            p[:, sl], relu, bias,
                        )
                prev = nxt

            # Output layer (identity + bias)
            po = psum_o.tile([kout, nb], f32, tag="po")
            for j in range(n_slots):
                sl = slice(int(offs[j]), int(offs[j]) + caps[j])
                for pi in range(nh):
                    nc.tensor.matmul(
                        po[:, sl],
                        wo_t[j][:, pi * kout : pi * kout + kout],
                        prev[:, pi * nb + int(offs[j]) : pi * nb + int(offs[j]) + caps[j]],
                        start=(pi == 0), stop=(pi == nh - 1),
                    )
            ot_t = pool.tile([kout, nb], f32, tag="ot")
            for j in range(n_slots):
                sl = slice(int(offs[j]), int(offs[j]) + caps[j])
                bias = bc_t[:kout, j * 8 + 6 : j * 8 + 7]
                act_op(ot_t[:, sl], po[:, sl], ident, bias)
            nc.sync.dma_start(ot_d[:, :], ot_t[:])

    nc.compile()
    return nc


def _get_program(caps, kin, hid, kout, w_dt_name):
    key = (caps, kin, hid, kout, w_dt_name)
    if key not in _PROGRAM_CACHE:
        _PROGRAM_CACHE[key] = _build_program(caps, kin, hid, kout, w_dt_name)
    return _PROGRAM_CACHE[key]


def _np_wdt(w_dt_name):
    if w_dt_name == "bf16":
        import ml_dtypes

        return np.dtype(ml_dtypes.bfloat16)
    return np.dtype(np.float32)


def _prep_core_inputs(plan, c, obs, maskbar, Wi, bi, W1, b1, W2, b2, W3, b3, Wo, bo,
                      w_dt_name):
    seq = maskbar.shape[1]
    kin = obs.shape[1]
    lobs = kin // seq
    hid = Wi.shape[3]
    kout = seq * Wo.shape[3]
    n_slots = plan["n_slots"]
    nb = plan["nb"]
    offs = plan["offs"]
    nk = (kin + 127) // 128
    nh = hid // 128
    wnp = _np_wdt(w_dt_name)

    xt = np.zeros((kin, nb), F32)
    mexp = np.zeros((kin, nb), F32)
    mbart = np.zeros((seq, nb), F32)
    bi_a = np.zeros((seq, n_slots * hid), F32)
    bc = np.zeros((128, n_slots * 8), F32)
    wi = np.zeros((n_slots, 128, nk * hid), F32)
    w = [np.zeros((n_slots, 128, nh * hid), F32) for _ in range(3)]
    wo = np.zeros((n_slots, 128, nh * kout), F32)

    for j in range(n_slots):
        r = plan["robot_at"][c][j]
        if r is None:
            continue
        rows = plan["rows"][c][j]
        n = len(rows)
        o0 = offs[j]
        if n:
            xt[:, o0 : o0 + n] = obs[rows].T
            mb = maskbar[rows]
            mexp[:, o0 : o0 + n] = np.repeat(mb, lobs, axis=1).T
            mbart[:, o0 : o0 + n] = mb.T
        wi[j] = _pack_kp(Wi[r].reshape(kin, hid))
        for li, W in enumerate((W1, W2, W3)):
            w[li][j] = _pack_kp(W[r])
        wo[j] = _pack_kp(Wo[r].transpose(1, 0, 2).reshape(hid, kout))
        bi_a[:, j * hid : (j + 1) * hid] = bi[r]
        for li, bvec in enumerate((b1[r], b2[r], b3[r])):
            for h in range(nh):
                bc[:, j * 8 + li * 2 + h] = bvec[h * 128 : (h + 1) * 128]
        bc[:kout, j * 8 + 6] = bo[r].reshape(-1)

    return {
        "xt": _pack_kp(xt).astype(wnp),
        "mexp": _pack_kp(mexp).astype(wnp),
        "mbart": mbart.astype(wnp),
        "bi": bi_a.astype(wnp),
        "bcols": bc,
        "wi": wi.astype(wnp),
        "w1": w[0].astype(wnp),
        "w2": w[1].astype(wnp),
        "w3": w[2].astype(wnp),
        "wo": wo.astype(wnp),
    }


def _unshard(plan, results, B, kout):
    out = np.zeros((B, kout), F32)
    offs = plan["offs"]
    for c in range(8):
        ot = results[c]["ot"]
        for j in range(plan["n_slots"]):
            rows = plan["rows"][c][j]
            if rows is None or len(rows) == 0:
                continue
            o0 = offs[j]
            out[rows] = np.asarray(ot[:, o0 : o0 + len(rows)], F32).T
    return out


def kernel(obs, obs_mask, unimal_ids, Wi, bi, W1, b1, W2, b2, W3, b3, Wo, bo,
           _runner=None, _w_dt=None):
    w_dt_name = _w_dt or W_DT
    obs = np.asarray(obs, F32)
    obs_mask = np.asarray(obs_mask)
    ids = np.asarray(unimal_ids).astype(np.int64)
    Wi, bi = np.asarray(Wi, F32), np.asarray(bi, F32)
    W1, b1 = np.asarray(W1, F32), np.asarray(b1, F32)
    W2, b2 = np.asarray(W2, F32), np.asarray(b2, F32)
    W3, b3 = np.asarray(W3, F32), np.asarray(b3, F32)
    Wo, bo = np.asarray(Wo, F32), np.asarray(bo, F32)

    B = obs.shape[0]
    n_robots = Wi.shape[0]
    seq, lobs, hid = Wi.shape[1], Wi.shape[2], Wi.shape[3]
    kin = seq * lobs
    kout = seq * Wo.shape[3]
    maskbar = 1.0 - obs_mask.astype(F32)

    plan = _plan(ids, n_robots)
    nc = _get_program(plan["caps"], kin, hid, kout, w_dt_name)

    in_maps = [
        _prep_core_inputs(plan, c, obs, maskbar, Wi, bi, W1, b1, W2, b2, W3, b3,
                          Wo, bo, w_dt_name)
        for c in range(8)
    ]

    if _runner is None:
        from concourse.bass_utils import run_bass_kernel_spmd

        res = run_bass_kernel_spmd(nc, in_maps, core_ids=list(range(8)))
        results = res.results
    else:
        results = _runner(nc, in_maps)

    return _unshard(plan, results, B, kout)


# revision 42
# speedup vs baseline: 2.2655x; 1.8383x over previous
"""Trainium2 Bass kernel for nn_MLPModel_70703751626902 (moe_routing).

Per-robot hypernetwork MLP: each of 1024 samples routes to one of 32
per-robot weight sets (input hypernet 624->256, three 256x256 hidden
layers, output hypernet 256->24).

Strategy (expert-parallel): group samples by robot on the host, shard
robots across the 8 cores (4 robots/core, one per "slot"), so every
core runs dense per-robot matmuls with only its own robots' weights
(~5.8MB/core instead of 46MB replicated). Activations stay transposed
([hidden, batch]) the whole way so each layer's PSUM output feeds the
next layer's moving operand directly. The obs mask is folded into the
transposed input with one elementwise multiply; the masked input bias
(maskbar @ bi) is a K=12 matmul into the same PSUM accumulation
group; all other biases ride along as per-partition bias operands of
the PSUM->SBUF relu/copy activation ops.

All DRAM tensors are packed host-side so every DMA moves >=2KB
contiguous runs per partition (128-partition-major packing of the
contraction dim).

Samples for slot j occupy columns [off_j, off_j + cap_j) where cap_j is
the max sample count over the 8 robots assigned to slot j (rounded up
to 8); robots are assigned to slots by descending count so padding
waste is small. All 8 cores run an identical program (SPMD).
"""

import numpy as np

F32 = np.float32

# matmul operand dtype: "f32" (exact, ~60us), "f32r" (fp32 bits, PE
# tf32-like fast path, rel err ~2e-4, ~44us), "bf16" (half DMA bytes,
# 1cyc/row PE, rel err ~3e-3, ~30us)
W_DT = "f32r"


def _plan(ids, n_robots):
    """Group samples by robot and assign robots to (core, slot)."""
    counts = np.bincount(ids, minlength=n_robots)
    order = np.argsort(-counts, kind="stable")
    n_slots = (n_robots + 7) // 8
    caps = []
    for j in range(n_slots):
        grp = order[8 * j : 8 * j + 8]
        m = int(counts[grp].max()) if len(grp) else 0
        caps.append(max(8, int(np.ceil(max(m, 1) / 8) * 8)))
    offs = np.concatenate([[0], np.cumsum(caps)]).astype(int)
    nb = int(offs[-1])
    assert nb <= 512, f"batch columns per core {nb} exceeds PSUM bank"
    rows = [[None] * n_slots for _ in range(8)]
    robot_at = [[None] * n_slots for _ in range(8)]
    for rank, robot in enumerate(order):
        j, c = rank // 8, rank % 8
        if j >= n_slots:
            break
        rows[c][j] = np.nonzero(ids == robot)[0]
        robot_at[c][j] = int(robot)
    return {
        "caps": tuple(caps),
        "offs": tuple(int(o) for o in offs),
        "nb": nb,
        "rows": rows,
        "robot_at": robot_at,
        "n_slots": n_slots,
    }


def _pack_kp(a, ncols=None):
    """[K, M] -> [128, ceil(K/128)*M]; col kt*M+m holds a[kt*128+p, m]."""
    k, m = a.shape
    nk = (k + 127) // 128
    out = np.zeros((128, nk * m), a.dtype)
    for kt in range(nk):
        ks = min(128, k - kt * 128)
        out[:ks, kt * m : kt * m + m] = a[kt * 128 : kt * 128 + ks, :]
    return out


_PROGRAM_CACHE = {}


def _build_program(caps, kin, seq, hid, kout, w_dt_name):
    import concourse.mybir as mybir
    import concourse.tile as tile
    from concourse import bacc

    f32 = mybir.dt.float32
    wdt = {"f32": f32, "f32r": mybir.dt.float32r, "bf16": mybir.dt.bfloat16}[w_dt_name]
    n_slots = len(caps)
    offs = np.concatenate([[0], np.cumsum(caps)]).astype(int)
    nb = int(offs[-1])
    # input-layer contraction: obs rows (kin) plus seq maskbar rows that
    # carry the masked input bias (bi rows ride in wi) — see host prep
    kaug = kin + seq
    nk = (kin + 127) // 128
    assert kaug <= nk * 128, "maskbar fold needs slack in the last chunk"
    klast = kaug - 128 * (nk - 1)
    nh = hid // 128  # hidden column halves

    import concourse.bass as bass_mod

    # Skip the framework's init-time all-engine barrier: it only
    # protects the const-AP memsets, which this kernel never reads
    # (every activation bias is a real SBUF column). Without it the
    # DMA queues start issuing ~4us earlier instead of waiting for the
    # slowest engine's program load. All data hazards are still covered
    # by Tile-generated semaphores, and the kernel-exit drain/barriers
    # are emitted after the patch is restored.
    _orig_barrier = bass_mod.Bass.all_engine_barrier
    bass_mod.Bass.all_engine_barrier = lambda self, *, sem_only=False: None
    try:
        nc = bacc.Bacc("TRN2", target_bir_lowering=False, debug=False, num_devices=8)
    finally:
        bass_mod.Bass.all_engine_barrier = _orig_barrier

    # xt and mexp share one tensor/DMA: [xt | mexp] along the free dim
    xtme_d = nc.dram_tensor("xtme", [128, 2 * nk * nb], wdt, kind="ExternalInput")
    bc_d = nc.dram_tensor("bcols", [128, n_slots * 8], f32, kind="ExternalInput")
    # weights packed slot-major in single tensors; DMAs pull column
    # ranges (groups of slots) so descriptor size and arrival order can
    # be tuned: slot 0 alone first (compute starts sooner), the rest in
    # bigger chunks (fatter descriptors, fewer serial dma_start issues)
    wiw = nk * hid  # wi columns per slot
    whw = 3 * nh * hid  # wh columns per slot
    if n_slots > 1:
        wi_groups = [(0, 1), (1, n_slots)]
        wh_groups = [(0, 2), (2, n_slots)] if n_slots > 2 else [(0, n_slots)]
    else:
        wi_groups = wh_groups = [(0, 1)]
    wi_d = nc.dram_tensor("wi", [128, n_slots * wiw], wdt, kind="ExternalInput")
    wh_d = nc.dram_tensor("wh", [128, n_slots * whw], wdt, kind="ExternalInput")
    wo_d = nc.dram_tensor(
        "wo", [128, n_slots * nh * kout], wdt, kind="ExternalInput"
    )
    ot_d = nc.dram_tensor("ot", [kout, nb], f32, kind="ExternalOutput")

    relu = mybir.ActivationFunctionType.Relu
    ident = mybir.ActivationFunctionType.Identity
    act_parity = [0]

    with tile.TileContext(nc) as tc:
        with (
            tc.tile_pool(name="sb", bufs=1) as pool,
            tc.tile_pool(name="ps", bufs=4, space="PSUM") as psum,
            tc.tile_pool(name="pso", bufs=1, space="PSUM") as psum_o,
        ):
            # DMA issue is ~0.7us of sequencer time per instruction and
            # transfers drain FIFO per queue, so each queue gets its
            # DMAs in need-order; the two HWDGE queues issue in parallel.
            # sync engine: weight groups in usage order
            wi_slot, wh_slot = {}, {}
            for g0, g1 in wi_groups:
                t = pool.tile([128, (g1 - g0) * wiw], wdt, tag=f"wig{g0}")
                nc.sync.dma_start(t[:], wi_d[:, g0 * wiw : g1 * wiw])
                for j in range(g0, g1):
                    wi_slot[j] = (t, (j - g0) * wiw)
            for g0, g1 in wh_groups:
                t = pool.tile([128, (g1 - g0) * whw], wdt, tag=f"whg{g0}")
                nc.sync.dma_start(t[:], wh_d[:, g0 * whw : g1 * whw])
                for j in range(g0, g1):
                    wh_slot[j] = (t, (j - g0) * whw)

            def wi_lhsT(j, kt, h, ks):
                t, base = wi_slot[j]
                o = base + kt * hid + h * 128
                return t[:ks, o : o + 128]

            def wh_lhsT(j, li, pi, h):
                t, base = wh_slot[j]
                o = base + li * nh * hid + pi * hid + h * 128
                return t[:, o : o + 128]

            # scalar engine: activations + small tensors + output weights
            xtme_t = pool.tile([128, 2 * nk * nb], wdt, tag="xtme")
            nc.scalar.dma_start(xtme_t[:], xtme_d[:, :])
            bc_t = pool.tile([128, n_slots * 8], f32, tag="bc")
            nc.scalar.dma_start(bc_t[:], bc_d[:, :])
            wo_t = pool.tile([128, n_slots * nh * kout], wdt, tag="wo")
            nc.scalar.dma_start(wo_t[:], wo_d[:, :])

            # masked transposed input (single packed multiply)
            xm_t = pool.tile([128, nk * nb], wdt, tag="xm")
            nc.vector.tensor_mul(
                xm_t[:], xtme_t[:, : nk * nb], xtme_t[:, nk * nb :]
            )

            def act_op(dst, src, func, bias):
                """PSUM->SBUF activation, alternating scalar/vector engines."""
                if act_parity[0] % 2 == 0:
                    nc.scalar.activation(dst, src, func, bias=bias)
                elif func is relu:
                    nc.vector.tensor_scalar(
                        dst, src, bias, 0.0,
                        mybir.AluOpType.add, mybir.AluOpType.max,
                    )
                else:
                    nc.vector.tensor_scalar(
                        dst, src, bias, None, mybir.AluOpType.add,
                    )
                act_parity[0] += 1

            # Layer 0: act0[h, b] = relu([xm; maskbar] @ [Wi; bi])
            act0 = pool.tile([128, nh * nb], wdt, tag="act0")
            p0 = [psum.tile([128, nb], f32, tag="ps", name=f"p0h{h}") for h in range(nh)]
            for j in range(n_slots):
                sl = slice(int(offs[j]), int(offs[j]) + caps[j])
                for h in range(nh):
                    for kt in range(nk):
                        ks = 128 if kt < nk - 1 else klast
                        nc.tensor.matmul(
                            p0[h][:, sl],
                            wi_lhsT(j, kt, h, ks),
                            xm_t[:ks, kt * nb + int(offs[j]) : kt * nb + int(offs[j]) + caps[j]],
                            start=(kt == 0), stop=(kt == nk - 1),
                        )
            zero_bias = bc_t[:, 7:8]  # unused bcols column, always zero
            for h in range(nh):
                act_op(act0[:, h * nb : (h + 1) * nb], p0[h][:, :], relu, zero_bias)

            # Hidden layers
            prev = act0
            for li in range(3):
                nxt = pool.tile([128, nh * nb], wdt, tag=f"act{li + 1}")
                for h in range(nh):
                    p = psum.tile([128, nb], f32, tag="ps")
                    for j in range(n_slots):
                        sl = slice(int(offs[j]), int(offs[j]) + caps[j])
                        for pi in range(nh):
                            nc.tensor.matmul(
                                p[:, sl],
                                wh_lhsT(j, li, pi, h),
                                prev[:, pi * nb + int(offs[j]) : pi * nb + int(offs[j]) + caps[j]],
                                start=(pi == 0), stop=(pi == nh - 1),
                            )
                    for j in range(n_slots):
                        sl = slice(int(offs[j]), int(offs[j]) + caps[j])
                        bias = bc_t[:, j * 8 + li * 2 + h : j * 8 + li * 2 + h + 1]
                        act_op(
                            nxt[:, h * nb + int(offs[j]) : h * nb + int(offs[j]) + caps[j]],
                            p[:, sl], relu, bias,
                        )
                prev = nxt

            # Output layer (identity + bias)
            po = psum_o.tile([kout, nb], f32, tag="po")
            for j in range(n_slots):
                sl = slice(int(offs[j]), int(offs[j]) + caps[j])
                for pi in range(nh):
                    w0 = (j * nh + pi) * kout
                    nc.tensor.matmul(
                        po[:, sl],
                        wo_t[:, w0 : w0 + kout],
                        prev[:, pi * nb + int(offs[j]) : pi * nb + int(offs[j]) + caps[j]],
                        start=(pi == 0), stop=(pi == nh - 1),
                    )
            # two out tiles so the first half's DMA can start while the
            # second half's bias-adds still run
            jh = (n_slots + 1) // 2
            mid = int(offs[jh])
            ot_a = pool.tile([kout, mid], f32, tag="ota")
            ot_b = pool.tile([kout, nb - mid], f32, tag="otb")
            for j in range(n_slots):
                sl = slice(int(offs[j]), int(offs[j]) + caps[j])
                bias = bc_t[:kout, j * 8 + 6 : j * 8 + 7]
                if j < jh:
                    dst = ot_a[:, int(offs[j]) : int(offs[j]) + caps[j]]
                else:
                    dst = ot_b[:, int(offs[j]) - mid : int(offs[j]) - mid + caps[j]]
                act_op(dst, po[:, sl], ident, bias)
                if j == jh - 1:
                    nc.sync.dma_start(ot_d[:, :mid], ot_a[:])
            nc.sync.dma_start(ot_d[:, mid:], ot_b[:])

    nc.compile()
    return nc


def _get_program(caps, kin, seq, hid, kout, w_dt_name):
    key = (caps, kin, seq, hid, kout, w_dt_name)
    if key not in _PROGRAM_CACHE:
        _PROGRAM_CACHE[key] = _build_program(caps, kin, seq, hid, kout, w_dt_name)
    return _PROGRAM_CACHE[key]


def _np_wdt(w_dt_name):
    if w_dt_name == "bf16":
        import ml_dtypes

        return np.dtype(ml_dtypes.bfloat16)
    return np.dtype(np.float32)


def _prep_core_inputs(plan, c, obs, maskbar, Wi, bi, W1, b1, W2, b2, W3, b3, Wo, bo,
                      w_dt_name):
    seq = maskbar.shape[1]
    kin = obs.shape[1]
    lobs = kin // seq
    hid = Wi.shape[3]
    kout = seq * Wo.shape[3]
    n_slots = plan["n_slots"]
    nb = plan["nb"]
    offs = plan["offs"]
    nk = (kin + 127) // 128
    nh = hid // 128
    wnp = _np_wdt(w_dt_name)

    kaug = kin + seq  # obs rows + maskbar rows (carry the input bias)
    xt = np.zeros((kaug, nb), F32)
    mexp = np.zeros((kaug, nb), F32)
    mexp[kin:, :] = 1.0
    bc = np.zeros((128, n_slots * 8), F32)
    wi = np.zeros((128, n_slots * nk * hid), F32)
    wh = np.zeros((128, n_slots * 3 * nh * hid), F32)
    wo = np.zeros((128, n_slots * nh * kout), F32)

    for j in range(n_slots):
        r = plan["robot_at"][c][j]
        if r is None:
            continue
        rows = plan["rows"][c][j]
        n = len(rows)
        o0 = offs[j]
        if n:
            xt[:kin, o0 : o0 + n] = obs[rows].T
            mb = maskbar[rows]
            mexp[:kin, o0 : o0 + n] = np.repeat(mb, lobs, axis=1).T
            xt[kin:, o0 : o0 + n] = mb.T
        o2 = j * nk * hid
        wi[:, o2 : o2 + nk * hid] = _pack_kp(
            np.vstack([Wi[r].reshape(kin, hid), bi[r]])
        )
        o2 = j * 3 * nh * hid
        for li, W in enumerate((W1, W2, W3)):
            wh[:, o2 + li * nh * hid : o2 + (li + 1) * nh * hid] = _pack_kp(W[r])
        wo[:, j * nh * kout : (j + 1) * nh * kout] = _pack_kp(
            Wo[r].transpose(1, 0, 2).reshape(hid, kout)
        )
        for li, bvec in enumerate((b1[r], b2[r], b3[r])):
            for h in range(nh):
                bc[:, j * 8 + li * 2 + h] = bvec[h * 128 : (h + 1) * 128]
        bc[:kout, j * 8 + 6] = bo[r].reshape(-1)

    return {
        "xtme": np.concatenate([_pack_kp(xt), _pack_kp(mexp)], axis=1).astype(wnp),
        "bcols": bc,
        "wi": wi.astype(wnp),
        "wh": wh.astype(wnp),
        "wo": wo.astype(wnp),
    }


def _unshard(plan, results, B, kout):
    out = np.zeros((B, kout), F32)
    offs = plan["offs"]
    for c in range(8):
        ot = results[c]["ot"]
        for j in range(plan["n_slots"]):
            rows = plan["rows"][c][j]
            if rows is None or len(rows) == 0:
                continue
            o0 = offs[j]
            out[rows] = np.asarray(ot[:, o0 : o0 + len(rows)], F32).T
    return out


def kernel(obs, obs_mask, unimal_ids, Wi, bi, W1, b1, W2, b2, W3, b3, Wo, bo,
           _runner=None, _w_dt=None):
    w_dt_name = _w_dt or W_DT
    obs = np.asarray(obs, F32)
    obs_mask = np.asarray(obs_mask)
    ids = np.asarray(unimal_ids).astype(np.int64)
    Wi, bi = np.asarray(Wi, F32), np.asarray(bi, F32)
    W1, b1 = np.asarray(W1, F32), np.asarray(b1, F32)
    W2, b2 = np.asarray(W2, F32), np.asarray(b2, F32)
    W3, b3 = np.asarray(W3, F32), np.asarray(b3, F32)
    Wo, bo = np.asarray(Wo, F32), np.asarray(bo, F32)

    B = obs.shape[0]
    n_robots = Wi.shape[0]
    seq, lobs, hid = Wi.shape[1], Wi.shape[2], Wi.shape[3]
    kin = seq * lobs
    kout = seq * Wo.shape[3]
    maskbar = 1.0 - obs_mask.astype(F32)

    plan = _plan(ids, n_robots)
    nc = _get_program(plan["caps"], kin, seq, hid, kout, w_dt_name)

    in_maps = [
        _prep_core_inputs(plan, c, obs, maskbar, Wi, bi, W1, b1, W2, b2, W3, b3,
                          Wo, bo, w_dt_name)
        for c in range(8)
    ]

    if _runner is None:
        from concourse.bass_utils import run_bass_kernel_spmd

        res = run_bass_kernel_spmd(nc, in_maps, core_ids=list(range(8)))
        results = res.results
    else:
        results = _runner(nc, in_maps)

    return _unshard(plan, results, B, kout)


# revision 45
# speedup vs baseline: 2.3653x; 1.0440x over previous
"""Trainium2 Bass kernel for nn_MLPModel_70703751626902 (moe_routing).

Per-robot hypernetwork MLP: each of 1024 samples routes to one of 32
per-robot weight sets (input hypernet 624->256, three 256x256 hidden
layers, output hypernet 256->24).

Strategy (expert-parallel): group samples by robot on the host, shard
robots across the 8 cores (4 robots/core, one per "slot"), so every
core runs dense per-robot matmuls with only its own robots' weights
(~5.8MB/core instead of 46MB replicated). Activations stay transposed
([hidden, batch]) the whole way so each layer's PSUM output feeds the
next layer's moving operand directly. The obs mask is folded into the
transposed input with one elementwise multiply; the masked input bias
(maskbar @ bi) rides inside the input-layer matmul itself (maskbar
rows appended to the input, bi rows appended to Wi); all other biases
ride along as per-partition bias operands of the PSUM->SBUF
relu/copy activation ops.

All DRAM tensors are packed host-side so every DMA moves >=2KB
contiguous runs per partition (128-partition-major packing of the
contraction dim).

Samples for slot j occupy columns [off_j, off_j + cap_j) where cap_j is
the max sample count over the 8 robots assigned to slot j (rounded up
to 8); robots are assigned to slots by descending count so padding
waste is small. All 8 cores run an identical program (SPMD).
"""

import numpy as np

F32 = np.float32

# matmul operand dtype: "f32" (exact, ~60us), "f32r" (fp32 bits, PE
# tf32-like fast path, rel err ~2e-4, ~44us), "bf16" (half DMA bytes,
# 1cyc/row PE, rel err ~3e-3, ~30us)
W_DT = "f32r"


def _plan(ids, n_robots):
    """Group samples by robot and assign robots to (core, slot)."""
    counts = np.bincount(ids, minlength=n_robots)
    order = np.argsort(-counts, kind="stable")
    n_slots = (n_robots + 7) // 8
    caps = []
    for j in range(n_slots):
        grp = order[8 * j : 8 * j + 8]
        m = int(counts[grp].max()) if len(grp) else 0
        caps.append(max(8, int(np.ceil(max(m, 1) / 8) * 8)))
    offs = np.concatenate([[0], np.cumsum(caps)]).astype(int)
    nb = int(offs[-1])
    assert nb <= 512, f"batch columns per core {nb} exceeds PSUM bank"
    rows = [[None] * n_slots for _ in range(8)]
    robot_at = [[None] * n_slots for _ in range(8)]
    for rank, robot in enumerate(order):
        j, c = rank // 8, rank % 8
        if j >= n_slots:
            break
        rows[c][j] = np.nonzero(ids == robot)[0]
        robot_at[c][j] = int(robot)
    return {
        "caps": tuple(caps),
        "offs": tuple(int(o) for o in offs),
        "nb": nb,
        "rows": rows,
        "robot_at": robot_at,
        "n_slots": n_slots,
    }


def _pack_kp(a, ncols=None):
    """[K, M] -> [128, ceil(K/128)*M]; col kt*M+m holds a[kt*128+p, m]."""
    k, m = a.shape
    nk = (k + 127) // 128
    out = np.zeros((128, nk * m), a.dtype)
    for kt in range(nk):
        ks = min(128, k - kt * 128)
        out[:ks, kt * m : kt * m + m] = a[kt * 128 : kt * 128 + ks, :]
    return out


_PROGRAM_CACHE = {}


def _build_program(caps, kin, seq, hid, kout, w_dt_name):
    import concourse.mybir as mybir
    import concourse.tile as tile
    from concourse import bacc

    f32 = mybir.dt.float32
    wdt = {"f32": f32, "f32r": mybir.dt.float32r, "bf16": mybir.dt.bfloat16}[w_dt_name]
    n_slots = len(caps)
    offs = np.concatenate([[0], np.cumsum(caps)]).astype(int)
    nb = int(offs[-1])
    # input-layer contraction: obs rows (kin) plus seq maskbar rows that
    # carry the masked input bias (bi rows ride in wi) — see host prep
    kaug = kin + seq
    nk = (kin + 127) // 128
    assert kaug <= nk * 128, "maskbar fold needs slack in the last chunk"
    klast = kaug - 128 * (nk - 1)
    nh = hid // 128  # hidden column halves

    import concourse.bass as bass_mod

    # Skip the framework's init-time all-engine barrier: it only
    # protects the const-AP memsets, which this kernel never reads
    # (every activation bias is a real SBUF column). Without it the
    # DMA queues start issuing ~4us earlier instead of waiting for the
    # slowest engine's program load. All data hazards are still covered
    # by Tile-generated semaphores, and the kernel-exit drain/barriers
    # are emitted after the patch is restored.
    _orig_barrier = bass_mod.Bass.all_engine_barrier
    bass_mod.Bass.all_engine_barrier = lambda self, *, sem_only=False: None
    try:
        nc = bacc.Bacc("TRN2", target_bir_lowering=False, debug=False, num_devices=8)
    finally:
        bass_mod.Bass.all_engine_barrier = _orig_barrier

    # xt and mexp interleaved in two pieces [xtA|meA|xtB|meB] so the
    # first input-layer chunks can start before the whole input lands
    nka = min(3, nk)
    nkb = nk - nka
    xtme_d = nc.dram_tensor("xtme", [128, 2 * nk * nb], wdt, kind="ExternalInput")
    bc_d = nc.dram_tensor("bcols", [128, n_slots * 8], f32, kind="ExternalInput")
    # weights packed slot-major in single tensors; DMAs pull column
    # ranges (groups of slots) so descriptor size and arrival order can
    # be tuned: slot 0 alone first (compute starts sooner), the rest in
    # bigger chunks (fatter descriptors, fewer serial dma_start issues)
    wiw = nk * hid  # wi columns per slot
    whw = 3 * nh * hid  # wh columns per slot
    # wi pieces in units of hid-column chunks (kt-granular): slot 0 is
    # split A/B so the first matmuls start before its tail arrives
    if n_slots > 1:
        wi_pieces = [(0, nka), (nka, nk)]
        if w_dt_name == "bf16":
            wi_pieces += [(nk, n_slots * nk)]
            wh_groups = [(0, 2), (2, n_slots)] if n_slots > 2 else [(0, n_slots)]
        else:
            wi_pieces += [(j * nk, (j + 1) * nk) for j in range(1, n_slots)]
            wh_groups = [(j, j + 1) for j in range(n_slots)]
    else:
        wi_pieces = [(0, nka), (nka, nk)] if nkb else [(0, nk)]
        wh_groups = [(0, 1)]
    wi_d = nc.dram_tensor("wi", [128, n_slots * wiw], wdt, kind="ExternalInput")
    wh_d = nc.dram_tensor("wh", [128, n_slots * whw], wdt, kind="ExternalInput")
    wo_d = nc.dram_tensor(
        "wo", [128, n_slots * nh * kout], wdt, kind="ExternalInput"
    )
    ot_d = nc.dram_tensor("ot", [kout, nb], f32, kind="ExternalOutput")

    relu = mybir.ActivationFunctionType.Relu
    ident = mybir.ActivationFunctionType.Identity
    act_parity = [0]

    with tile.TileContext(nc) as tc:
        with (
            tc.tile_pool(name="sb", bufs=1) as pool,
            tc.tile_pool(name="ps", bufs=4, space="PSUM") as psum,
            tc.tile_pool(name="pso", bufs=1, space="PSUM") as psum_o,
        ):
            # DMA issue is ~0.7us of sequencer time per instruction and
            # transfers drain FIFO per queue, so each queue gets its
            # DMAs in need-order; the two HWDGE queues issue in parallel.
            # sync engine: weight pieces in usage order
            wi_chunk, wh_slot = {}, {}
            for c0, c1 in wi_pieces:
                t = pool.tile([128, (c1 - c0) * hid], wdt, tag=f"wig{c0}")
                nc.sync.dma_start(t[:], wi_d[:, c0 * hid : c1 * hid])
                for c in range(c0, c1):
                    wi_chunk[c] = (t, (c - c0) * hid)
            for g0, g1 in wh_groups:
                t = pool.tile([128, (g1 - g0) * whw], wdt, tag=f"whg{g0}")
                nc.sync.dma_start(t[:], wh_d[:, g0 * whw : g1 * whw])
                for j in range(g0, g1):
                    wh_slot[j] = (t, (j - g0) * whw)

            def wi_lhsT(j, kt, h, ks):
                t, base = wi_chunk[j * nk + kt]
                o = base + h * 128
                return t[:ks, o : o + 128]

            def wh_lhsT(j, li, pi, h):
                t, base = wh_slot[j]
                o = base + li * nh * hid + pi * hid + h * 128
                return t[:, o : o + 128]

            # scalar engine: inputs (two pieces) + small tensors + wo
            xtme_a = pool.tile([128, 2 * nka * nb], wdt, tag="xtmea")
            nc.scalar.dma_start(xtme_a[:], xtme_d[:, : 2 * nka * nb])
            if nkb:
                xtme_b = pool.tile([128, 2 * nkb * nb], wdt, tag="xtmeb")
                nc.scalar.dma_start(xtme_b[:], xtme_d[:, 2 * nka * nb :])
            bc_t = pool.tile([128, n_slots * 8], f32, tag="bc")
            nc.scalar.dma_start(bc_t[:], bc_d[:, :])
            wo_t = pool.tile([128, n_slots * nh * kout], wdt, tag="wo")
            nc.scalar.dma_start(wo_t[:], wo_d[:, :])

            # masked transposed input (one packed multiply per piece)
            xm_a = pool.tile([128, nka * nb], wdt, tag="xma")
            nc.vector.tensor_mul(
                xm_a[:], xtme_a[:, : nka * nb], xtme_a[:, nka * nb :]
            )
            if nkb:
                xm_b = pool.tile([128, nkb * nb], wdt, tag="xmb")
                nc.vector.tensor_mul(
                    xm_b[:], xtme_b[:, : nkb * nb], xtme_b[:, nkb * nb :]
                )

            def xm_rhs(kt, ks, c0, w):
                if kt < nka:
                    return xm_a[:ks, kt * nb + c0 : kt * nb + c0 + w]
                return xm_b[:ks, (kt - nka) * nb + c0 : (kt - nka) * nb + c0 + w]

            def act_op(dst, src, func, bias):
                """PSUM->SBUF activation, alternating scalar/vector engines."""
                if act_parity[0] % 2 == 0:
                    nc.scalar.activation(dst, src, func, bias=bias)
                elif func is relu:
                    nc.vector.tensor_scalar(
                        dst, src, bias, 0.0,
                        mybir.AluOpType.add, mybir.AluOpType.max,
                    )
                else:
                    nc.vector.tensor_scalar(
                        dst, src, bias, None, mybir.AluOpType.add,
                    )
                act_parity[0] += 1

            # Layer 0: act0[h, b] = relu([xm; maskbar] @ [Wi; bi])
            act0 = pool.tile([128, nh * nb], wdt, tag="act0")
            p0 = [psum.tile([128, nb], f32, tag="ps", name=f"p0h{h}") for h in range(nh)]
            for j in range(n_slots):
                sl = slice(int(offs[j]), int(offs[j]) + caps[j])
                for h in range(nh):
                    for kt in range(nk):
                        ks = 128 if kt < nk - 1 else klast
                        nc.tensor.matmul(
                            p0[h][:, sl],
                            wi_lhsT(j, kt, h, ks),
                            xm_rhs(kt, ks, int(offs[j]), caps[j]),
                            start=(kt == 0), stop=(kt == nk - 1),
                        )
            zero_bias = bc_t[:, 7:8]  # unused bcols column, always zero
            for h in range(nh):
                act_op(act0[:, h * nb : (h + 1) * nb], p0[h][:, :], relu, zero_bias)

            # Hidden layers
            prev = act0
            for li in range(3):
                nxt = pool.tile([128, nh * nb], wdt, tag=f"act{li + 1}")
                for h in range(nh):
                    p = psum.tile([128, nb], f32, tag="ps")
                    for j in range(n_slots):
                        sl = slice(int(offs[j]), int(offs[j]) + caps[j])
                        for pi in range(nh):
                            nc.tensor.matmul(
                                p[:, sl],
                                wh_lhsT(j, li, pi, h),
                                prev[:, pi * nb + int(offs[j]) : pi * nb + int(offs[j]) + caps[j]],
                                start=(pi == 0), stop=(pi == nh - 1),
                            )
                    for j in range(n_slots):
                        sl = slice(int(offs[j]), int(offs[j]) + caps[j])
                        bias = bc_t[:, j * 8 + li * 2 + h : j * 8 + li * 2 + h + 1]
                        act_op(
                            nxt[:, h * nb + int(offs[j]) : h * nb + int(offs[j]) + caps[j]],
                            p[:, sl], relu, bias,
                        )
                prev = nxt

            # Output layer (identity + bias)
            po = psum_o.tile([kout, nb], f32, tag="po")
            for j in range(n_slots):
                sl = slice(int(offs[j]), int(offs[j]) + caps[j])
                for pi in range(nh):
                    w0 = (j * nh + pi) * kout
                    nc.tensor.matmul(
                        po[:, sl],
                        wo_t[:, w0 : w0 + kout],
                        prev[:, pi * nb + int(offs[j]) : pi * nb + int(offs[j]) + caps[j]],
                        start=(pi == 0), stop=(pi == nh - 1),
                    )
            # two out tiles so the first half's DMA can start while the
            # second half's bias-adds still run
            jh = (n_slots + 1) // 2
            mid = int(offs[jh])
            ot_a = pool.tile([kout, mid], f32, tag="ota")
            ot_b = pool.tile([kout, nb - mid], f32, tag="otb")
            for j in range(n_slots):
                sl = slice(int(offs[j]), int(offs[j]) + caps[j])
                bias = bc_t[:kout, j * 8 + 6 : j * 8 + 7]
                if j < jh:
                    dst = ot_a[:, int(offs[j]) : int(offs[j]) + caps[j]]
                else:
                    dst = ot_b[:, int(offs[j]) - mid : int(offs[j]) - mid + caps[j]]
                act_op(dst, po[:, sl], ident, bias)
                if j == jh - 1:
                    nc.sync.dma_start(ot_d[:, :mid], ot_a[:])
            nc.sync.dma_start(ot_d[:, mid:], ot_b[:])

    nc.compile()
    return nc


def _get_program(caps, kin, seq, hid, kout, w_dt_name):
    key = (caps, kin, seq, hid, kout, w_dt_name)
    if key not in _PROGRAM_CACHE:
        _PROGRAM_CACHE[key] = _build_program(caps, kin, seq, hid, kout, w_dt_name)
    return _PROGRAM_CACHE[key]


def _np_wdt(w_dt_name):
    if w_dt_name == "bf16":
        import ml_dtypes

        return np.dtype(ml_dtypes.bfloat16)
    return np.dtype(np.float32)


def _prep_core_inputs(plan, c, obs, maskbar, Wi, bi, W1, b1, W2, b2, W3, b3, Wo, bo,
                      w_dt_name):
    seq = maskbar.shape[1]
    kin = obs.shape[1]
    lobs = kin // seq
    hid = Wi.shape[3]
    kout = seq * Wo.shape[3]
    n_slots = plan["n_slots"]
    nb = plan["nb"]
    offs = plan["offs"]
    nk = (kin + 127) // 128
    nh = hid // 128
    wnp = _np_wdt(w_dt_name)

    kaug = kin + seq  # obs rows + maskbar rows (carry the input bias)
    xt = np.zeros((kaug, nb), F32)
    mexp = np.zeros((kaug, nb), F32)
    mexp[kin:, :] = 1.0
    bc = np.zeros((128, n_slots * 8), F32)
    wi = np.zeros((128, n_slots * nk * hid), F32)
    wh = np.zeros((128, n_slots * 3 * nh * hid), F32)
    wo = np.zeros((128, n_slots * nh * kout), F32)

    for j in range(n_slots):
        r = plan["robot_at"][c][j]
        if r is None:
            continue
        rows = plan["rows"][c][j]
        n = len(rows)
        o0 = offs[j]
        if n:
            xt[:kin, o0 : o0 + n] = obs[rows].T
            mb = maskbar[rows]
            mexp[:kin, o0 : o0 + n] = np.repeat(mb, lobs, axis=1).T
            xt[kin:, o0 : o0 + n] = mb.T
        o2 = j * nk * hid
        wi[:, o2 : o2 + nk * hid] = _pack_kp(
            np.vstack([Wi[r].reshape(kin, hid), bi[r]])
        )
        o2 = j * 3 * nh * hid
        for li, W in enumerate((W1, W2, W3)):
            wh[:, o2 + li * nh * hid : o2 + (li + 1) * nh * hid] = _pack_kp(W[r])
        wo[:, j * nh * kout : (j + 1) * nh * kout] = _pack_kp(
            Wo[r].transpose(1, 0, 2).reshape(hid, kout)
        )
        for li, bvec in enumerate((b1[r], b2[r], b3[r])):
            for h in range(nh):
                bc[:, j * 8 + li * 2 + h] = bvec[h * 128 : (h + 1) * 128]
        bc[:kout, j * 8 + 6] = bo[r].reshape(-1)

    nka = min(3, nk)
    xtp, mep = _pack_kp(xt), _pack_kp(mexp)
    xtme = np.concatenate(
        [xtp[:, : nka * nb], mep[:, : nka * nb],
         xtp[:, nka * nb :], mep[:, nka * nb :]], axis=1,
    )
    return {
        "xtme": xtme.astype(wnp),
        "bcols": bc,
        "wi": wi.astype(wnp),
        "wh": wh.astype(wnp),
        "wo": wo.astype(wnp),
    }


def _unshard(plan, results, B, kout):
    out = np.zeros((B, kout), F32)
    offs = plan["offs"]
    for c in range(8):
        ot = results[c]["ot"]
        for j in range(plan["n_slots"]):
            rows = plan["rows"][c][j]
            if rows is None or len(rows) == 0:
                continue
            o0 = offs[j]
            out[rows] = np.asarray(ot[:, o0 : o0 + len(rows)], F32).T
    return out


def kernel(obs, obs_mask, unimal_ids, Wi, bi, W1, b1, W2, b2, W3, b3, Wo, bo,
           _runner=None, _w_dt=None):
    w_dt_name = _w_dt or W_DT
    obs = np.asarray(obs, F32)
    obs_mask = np.asarray(obs_mask)
    ids = np.asarray(unimal_ids).astype(np.int64)
    Wi, bi = np.asarray(Wi, F32), np.asarray(bi, F32)
    W1, b1 = np.asarray(W1, F32), np.asarray(b1, F32)
    W2, b2 = np.asarray(W2, F32), np.asarray(b2, F32)
    W3, b3 = np.asarray(W3, F32), np.asarray(b3, F32)
    Wo, bo = np.asarray(Wo, F32), np.asarray(bo, F32)

    B = obs.shape[0]
    n_robots = Wi.shape[0]
    seq, lobs, hid = Wi.shape[1], Wi.shape[2], Wi.shape[3]
    kin = seq * lobs
    kout = seq * Wo.shape[3]
    maskbar = 1.0 - obs_mask.astype(F32)

    plan = _plan(ids, n_robots)
    nc = _get_program(plan["caps"], kin, seq, hid, kout, w_dt_name)

    in_maps = [
        _prep_core_inputs(plan, c, obs, maskbar, Wi, bi, W1, b1, W2, b2, W3, b3,
                          Wo, bo, w_dt_name)
        for c in range(8)
    ]

    if _runner is None:
        from concourse.bass_utils import run_bass_kernel_spmd

        res = run_bass_kernel_spmd(nc, in_maps, core_ids=list(range(8)))
        results = res.results
    else:
        results = _runner(nc, in_maps)

    return _unshard(plan, results, B, kout)
